# revision 25
# baseline (speedup 1.0000x reference)
"""Trainium2 kernel for nn_COSSIMMLP (gnn_message_passing).

reference semantics:
    src = prop_state[b, mask[...,0]]; dst = prop_state[b, mask[...,1]]
    vals = sigmoid(cossim(src, dst))          # [B, E]
    adj[b, i, j] = vals; adj[b, j, i] = vals  # dense [B, N, N]

Every scatter write at position (r, c) carries the identical value
sigmoid(cos(s_r, s_c)), and adj is exactly symmetric with zeros at
non-edge positions.  The device therefore computes only the folded
half-gram G = (4*S_hat)(4*S_hat)^T in fp8 (so each slab entry holds
16*cos) and ships it back raw; the host gathers the ~E edge entries,
applies the exact sigmoid to just those, and scatters them into a
zeroed dense adjacency.  Non-edges are exact zeros, so no mask tensor
ever crosses the DMA, and no engine touches a 4M-entry sigmoid.

8 cores = 4 batches x 2 LHS-tile-halves, node order rolled per core by
2048*h so one SPMD program serves all cores.  In 128-row tile
coordinates the core owning LHS tiles m=0..15 computes gram blocks
(m, m+d) for ring distance d=0..15 as a [2048, 2048] slab plus d=16 as
a separate [128, 16*128] strip (strip layout: partition p, col t*128+c
holds gram row t*128+p, ring-16 col c; computed redundantly by both
cores of a pair).  fp8 e4m3 holds 16*cos to ~2% which perturbs
sigmoid(cos) by only ~6e-4 relative (cos ~ N(0,1/256) for D=256), far
inside the 2e-2 gate.

Engine budget per core: PE streams 16*2048+2048 DoubleRow fp8 columns,
PSUM->SBUF fp8 cast-copies on the only two PSUM-capable engines (ACT
0.83ns/col, DVE 1.04ns/col, ~260/158ns per-instruction overhead) are
the critical path at ~20us parallel.  Empirically the PE must keep a
FINE-grained matmul rhythm: coarse PSUM tiles (2048/1024 cols, 2-4 in
flight) opened periodic matmul gaps that dropped the PE to its
~1.2GHz p-state and regressed 25-50%.  So PSUM is ONE [128,4096] f32
tile (all 8 banks) used as a ring of eight 512-col slices: matmuls
write single slices back-to-back at 2.4GHz, while copies drain
adjacent slice PAIRS (1024 cols, halving instruction overhead),
greedy-balanced across ACT/DVE; m=0 uses two 512-col singles to cut
the pipeline ramp.  The d=16 strip is computed as two items of eight
small matmuls (batching what would be sixteen overhead-dominated
128-col copies) interleaved mid-loop, landing in a partition-major
outc.  Input is host-normalized fp8 in partition-contiguous layout,
loaded as four pieces on the two HWDGE queues (sync/scalar) — the
software-DGE gpsimd queue starts transfers ~4us late — so m=0's
first chunk unblocks ~9us in.
"""

import numpy as np
import ml_dtypes

B, N, D, E = 4, 4096, 256, 131072
P = 128              # partitions
MT = 16              # LHS tiles per core (2048 rows)
ROWS = MT * P        # 2048
MCOLS = 16 * P       # 2048 main cols per slab row-tile (ring distance 0..15)
COLS = 17 * P        # 2176 incl. the d=16 strip
EPS = 1e-8
ACT_COLS = 1088      # ACT/DVE split of each 2048-col copy (balances
DVE_COLS = MCOLS - ACT_COLS  # 0.833ns/col+260ns vs 1.042ns/col+158ns)

_prog = None


def _build_program():
    import concourse.tile as tile
    from concourse import bacc, mybir

    f32 = mybir.dt.float32
    fp8 = mybir.dt.float8e4
    DR = mybir.MatmulPerfMode.DoubleRow

    nc = bacc.Bacc("TRN2", target_bir_lowering=False, debug=False)
    # st[p, kt*N + n] = 4*s_hat[node n, dim kt*128+p]: one contiguous 8KB
    # line per partition, DoubleRow k-major
    st_in = nc.dram_tensor("st", [P, 2 * N], fp8, kind="ExternalInput")
    outb = nc.dram_tensor("outb", [ROWS, MCOLS], fp8, kind="ExternalOutput")
    outc = nc.dram_tensor("outc", [P, MT * P], fp8, kind="ExternalOutput")

    st_r = st_in.rearrange("p (kt n) -> p kt n", kt=2)

    with tile.TileContext(nc) as tc:
        with (
            tc.tile_pool(name="const", bufs=1) as cpool,
            tc.tile_pool(name="outp", bufs=4) as outp,
        ):
            st = cpool.tile([P, 2, N], fp8)
            # input pieces on the two HWDGE queues only (the software-DGE
            # gpsimd queue starts transfers ~4us late): a small first piece
            # unblocks m=0's first chunk ~9.3us in, the rest lands by ~11us
            nc.sync.dma_start(out=st[:, :, 0:512], in_=st_r[:, :, 0:512])
            nc.scalar.dma_start(out=st[:, :, 512:1024], in_=st_r[:, :, 512:1024])
            nc.scalar.dma_start(out=st[:, :, 1024:1536], in_=st_r[:, :, 1024:1536])
            nc.sync.dma_start(out=st[:, :, 1536:2560], in_=st_r[:, :, 1536:2560])
            nc.sync.dma_start(out=st[:, :, 2560:N], in_=st_r[:, :, 2560:N])

            # greedy-balanced ACT/DVE assignment per 1024-col copy
            cost = {"s": 0.0, "v": 0.0}
            rate = {"s": 1113.0, "v": 1225.0}

            def emit_copy(ot_slice, ps_slice):
                e = min(cost, key=lambda k: cost[k] + rate[k])
                cost[e] += rate[e]
                if e == "s":
                    nc.scalar.copy(out=ot_slice, in_=ps_slice)
                else:
                    nc.vector.tensor_copy(out=ot_slice, in_=ps_slice)

            with tc.tile_pool(name="mmps", bufs=1, space="PSUM") as mmps:
                # all 8 PSUM banks as ONE tile used as a ring of 8 512-col
                # slices: matmuls write single slices (fine-grained PE
                # rhythm), copies read adjacent PAIRS in one instruction
                # (halves the ~260ns per-copy overhead); slice-level
                # dependency tracking handles the ring reuse
                psall = mmps.tile([P, 8 * 512], f32)
                ring = [0]

                def next_slice():
                    r = ring[0]
                    ring[0] = (r + 1) % 8
                    return psall[:, r * 512 : (r + 1) * 512], r

                def emit_main(m, singles=0):
                    # `singles`: leading chunks copied one-by-one (512 cols)
                    # so the very first copy isn't gated on two matmuls —
                    # used for m=0 to cut ~1.2us off the pipeline ramp
                    base = m * P
                    lhs = st[:, :, base : base + P]
                    ot = outp.tile([P, MCOLS], fp8, tag="ot")
                    q = 0
                    while q < 4:
                        pair = q >= singles
                        r0 = None
                        for qq in range(q, q + (2 if pair else 1)):
                            ps, r = next_slice()
                            if r0 is None:
                                r0 = r
                            nc.tensor.matmul(
                                ps,
                                lhsT=lhs,
                                rhs=st[
                                    :, :, base + qq * 512 : base + (qq + 1) * 512
                                ],
                                perf_mode=DR,
                                start=True,
                                stop=True,
                            )
                        w = 2 if pair else 1
                        emit_copy(
                            ot[:, q * 512 : (q + w) * 512],
                            psall[:, r0 * 512 : (r0 + w) * 512],
                        )
                        q += w
                    nc.sync.dma_start(out=outb[base : base + P, :], in_=ot[:])

                def emit_strip(gp):
                    # d=16 strip groups (2g, 2g+1): 8 small matmuls into an
                    # adjacent slice pair -> one 1024-col copy -> one DMA
                    otc = outp.tile([P, 1024], fp8, tag="otc")
                    r0 = None
                    for sub in range(2):
                        ps, r = next_slice()
                        if r0 is None:
                            r0 = r
                        for k in range(4):
                            base = (8 * gp + 4 * sub + k) * P
                            nc.tensor.matmul(
                                ps[:, k * P : (k + 1) * P],
                                lhsT=st[:, :, base : base + P],
                                rhs=st[:, :, base + MCOLS : base + COLS],
                                perf_mode=DR,
                                start=True,
                                stop=True,
                            )
                    emit_copy(otc[:], psall[:, r0 * 512 : (r0 + 2) * 512])
                    nc.sync.dma_start(
                        out=outc[:, gp * 1024 : (gp + 1) * 1024], in_=otc[:]
                    )

                # m0/m1 lead with single-chunk copies: the copy engines are
                # input-bound early, so finer quanta start sooner and fill
                # the piece-arrival gaps; the final item is a strip so the
                # run ends on a 1KB DMA instead of a 2KB one
                emit_main(0, singles=4)
                emit_main(1, singles=2)
                for m in range(2, MT):
                    emit_main(m)
                    if m == 5:
                        emit_strip(0)
                emit_strip(1)

    nc.compile()
    return nc


def _host_prep(prop_state, mask):
    prop = np.asarray(prop_state, dtype=np.float32)
    nrm = np.sqrt(np.einsum("bnd,bnd->bn", prop, prop))
    shat4 = prop * (4.0 / np.maximum(nrm, EPS))[..., None]
    shat4 = shat4.astype(ml_dtypes.float8_e4m3)  # [B, N, D]

    in_maps = []
    for c in range(8):
        b, h = divmod(c, 2)
        r = ROWS * h
        rolled = shat4[b] if r == 0 else np.roll(shat4[b], -r, axis=0)
        # [N, D] -> [P, 2*N] partition-contiguous DoubleRow k-major
        st = np.ascontiguousarray(
            rolled.T.reshape(2, P, N).transpose(1, 0, 2).reshape(P, 2 * N)
        )
        in_maps.append({"st": st})
    return in_maps


def _assemble(results, mask):
    mk = np.asarray(mask)
    out = np.zeros((B, N, N), dtype=np.float32)
    for b in range(B):
        i = mk[b, :, 0].astype(np.int64)
        j = mk[b, :, 1].astype(np.int64)
        mains = [results[2 * b]["outb"], results[2 * b + 1]["outb"]]
        strips = [results[2 * b]["outc"], results[2 * b + 1]["outc"]]
        val = np.empty(E, dtype=np.float64)
        found = np.zeros(E, dtype=bool)
        for x, y in ((i, j), (j, i)):
            for h in (0, 1):
                xr = (x - ROWS * h) % N
                yr = (y - ROWS * h) % N
                cc = yr - (xr >> 7 << 7)
                sel = ~found & (xr < ROWS)
                okm = sel & (cc >= 0) & (cc < MCOLS)
                idx = np.nonzero(okm)[0]
                if idx.size:
                    val[idx] = mains[h][xr[idx], cc[idx]].astype(np.float64)
                    found[idx] = True
                oks = sel & (cc >= MCOLS) & (cc < COLS)
                idx = np.nonzero(oks)[0]
                if idx.size:
                    xi = xr[idx]
                    val[idx] = strips[h][
                        xi & 127, (xi >> 7 << 7) + cc[idx] - MCOLS
                    ].astype(np.float64)
                    found[idx] = True
        assert found.all()
        v = 1.0 / (1.0 + np.exp(-val / 16.0))
        v[i == j] = 0.7310585786300049  # sigmoid(1): self-cossim is exactly 1
        v = v.astype(np.float32)
        out[b, i, j] = v
        out[b, j, i] = v
    return out


def kernel(prop_state, mask):
    from concourse.bass_utils import run_bass_kernel_spmd

    global _prog
    if _prog is None:
        _prog = _build_program()
    in_maps = _host_prep(prop_state, mask)
    res = run_bass_kernel_spmd(_prog, in_maps, core_ids=list(range(8)))
    return _assemble(res.results, mask)
